# revision 1
# baseline (speedup 1.0000x reference)
"""FP8 linear kernel for Trainium2, 8 NeuronCores.

y = (quant_e4m3fn(x) @ W.T) * (x_inv_scale * w_scale), output bf16.
x [8192, 4096] f32, W [14336, 4096] fp8 e4m3fn, w_scale f32 scalar.

Sharding: 2 token-halves x 4 out_feature-quarters (tensor parallel on
out_features per the hint, plus 2-way data parallel on tokens). Each
core computes y[half, quarter] = [4096, 3584]; the host assembles the
full output.

Exactness strategy: TRN fp8_e4m3 saturates at +-240 (vs OCP e4m3fn's
448), so both operands are staged/quantized at HALF scale (values <=
224), where the two formats agree bit-for-bit, and the dequant factor
carries the compensating 4x. Halving is exact for fp8 normals, so the
kernel reproduces the reference quantization exactly (modulo the fp8
subnormal tail and fp32 summation order).

The global activation amax is computed cooperatively: each core reduces
1/8 of x locally, then an AllReduce(max) collective merges the 8
partial maxima on-device (measured <50us, hidden under the weight-load
DMA).

Matmul: DoubleRow fp8 (2 k-planes per PE cell, K=256 per stationary),
x_qT stationary / W^T moving, PSUM [128 tokens, 4x448] accumulated over
K=4096, double-buffered across the two 1792-wide output halves so
eviction (dequant-scale multiply + bf16 cast on DVE) never stalls the
PE. Host pre-packs x and W^T into partition-major tile layouts so every
DMA is a single fully-contiguous transfer.
"""

import contextlib
import os
import sys

for _p in ("/opt/trn_rl_repo", "/root/.axon_site/_ro/trn_rl_repo"):
    if os.path.isdir(_p) and _p not in sys.path:
        sys.path.insert(0, _p)

import ml_dtypes
import numpy as np

import concourse.bass_isa as bass_isa
import concourse.mybir as mybir
import concourse.tile as tile
from concourse import bacc
from concourse.bass import ds, ts
from concourse.bass_utils import run_bass_kernel_spmd

F32 = mybir.dt.float32
BF16 = mybir.dt.bfloat16
FP8 = mybir.dt.float8e4
FP8_NP = mybir.dt.np(FP8)
E4M3FN = ml_dtypes.float8_e4m3fn

TOKENS, DIN, DOUT = 8192, 4096, 14336
P = 128
KO = DIN // P               # 32 k-subtiles of 128
N_CORES = 8
TOK_WAYS, OF_WAYS = 2, 4    # sharding grid
T_SH = TOKENS // TOK_WAYS   # 4096 tokens per core
OF_SH = DOUT // OF_WAYS     # 3584 out features per core
TT = T_SH // P              # 32 token tiles per core
AT = TT // OF_WAYS          # 8 amax tiles per core (1/8 of x total)
NB = 448                    # psum bank free dim; 4 banks = 1792 = OF_SH/2
OF_HALF = OF_SH // 2        # 1792

# 'doublerow' (fast fp8) or 'plain' (exact fp8 at bf16 speed, ~2x slower)
PERF = os.environ.get("FP8LIN_PERF", "doublerow")

_CACHE = {}


def _phase_b(nc, work, xqp, psum, xh, y, wres, qv, mf, dr):
    """Quantize + matmul + scaled eviction over all token tiles."""
    for t in range(TT):
        xf = work.tile([P, KO, P], F32, tag="xf")
        nc.sync.dma_start(out=xf[:], in_=xh[t])
        xq = xqp.tile([P, KO, P], FP8, tag="xq")
        nc.scalar.activation(
            xq[:], xf[:], mybir.ActivationFunctionType.Copy, scale=qv[:]
        )
        for h2 in range(2):
            ps = [psum.tile([P, NB], F32, name=f"ps{i}") for i in range(4)]
            if dr is not None:
                for k2 in range(KO // 2):
                    lhsT = xq[:, 2 * k2 : 2 * k2 + 2, :]
                    for nb in range(4):
                        nc.tensor.matmul(
                            ps[nb][:],
                            lhsT,
                            wres[:, 2 * k2 : 2 * k2 + 2,
                                 ds(h2 * OF_HALF + nb * NB, NB)],
                            start=(k2 == 0),
                            stop=(k2 == KO // 2 - 1),
                            perf_mode=dr,
                        )
            else:
                for k in range(KO):
                    lhsT = xq[:, k, :]
                    for nb in range(4):
                        nc.tensor.matmul(
                            ps[nb][:],
                            lhsT,
                            wres[:, k, ds(h2 * OF_HALF + nb * NB, NB)],
                            start=(k == 0),
                            stop=(k == KO - 1),
                        )
            yt = work.tile([P, 4, NB], BF16, tag="yt")
            for nb in range(4):
                nc.vector.tensor_scalar_mul(yt[:, nb, :], ps[nb][:], mf[:])
            nc.sync.dma_start(
                out=y[ts(t, P), ds(h2 * OF_HALF, OF_HALF)],
                in_=yt[:],
            )


def _build_module(reps=1, phase_a=True):
    """reps>1 loops phase B on-device (slope benchmarking only);
    phase_a=False substitutes constant scales (benchmarking only)."""
    key = ("module", PERF, reps, phase_a)
    if key in _CACHE:
        return _CACHE[key]

    nc = bacc.Bacc(None, target_bir_lowering=False, debug=True, num_devices=N_CORES)
    xh = nc.declare_dram_parameter("xh", [TT, P, KO, P], F32, isOutput=False)
    xa = nc.declare_dram_parameter("xa", [AT, P, KO, P], F32, isOutput=False)
    w3 = nc.declare_dram_parameter("w3", [P, KO, OF_SH], FP8, isOutput=False)
    ws = nc.declare_dram_parameter("ws", [P, 1], F32, isOutput=False)
    y = nc.declare_dram_parameter("y", [T_SH, OF_SH], BF16, isOutput=True)
    cc_in = nc.dram_tensor("cc_in", [1], F32)
    cc_out = nc.dram_tensor("cc_out", [1], F32, addr_space="Shared")

    dr = mybir.MatmulPerfMode.DoubleRow if PERF == "doublerow" else None

    with tile.TileContext(nc) as tc:
        with (
            tc.tile_pool(name="const", bufs=1) as const,
            tc.tile_pool(name="work", bufs=3) as work,
            tc.tile_pool(name="xqp", bufs=2) as xqp,
            tc.tile_pool(name="psum", bufs=2, space="PSUM") as psum,
        ):
            # resident weight [128, KO, OF_SH] fp8 (14.3 MB)
            wres = const.tile([P, KO, OF_SH], FP8)
            for i in range(4):
                nc.sync.dma_start(
                    out=wres[:, ts(i, KO // 4), :], in_=w3[:, ts(i, KO // 4), :]
                )
            wssb = const.tile([P, 1], F32)
            nc.sync.dma_start(out=wssb[:], in_=ws[:])

            if phase_a:
                # ---- phase A: global amax of x ----
                partials = const.tile([P, AT], F32)
                for i in range(AT):
                    xt = work.tile([P, KO, P], F32, tag="xf")
                    nc.sync.dma_start(out=xt[:], in_=xa[i])
                    nc.vector.tensor_reduce(
                        partials[:, i : i + 1],
                        xt[:],
                        axis=mybir.AxisListType.XY,
                        op=mybir.AluOpType.max,
                        apply_absolute_value=True,
                    )
                loc = const.tile([P, 1], F32)
                nc.vector.tensor_reduce(
                    loc[:], partials[:], axis=mybir.AxisListType.X,
                    op=mybir.AluOpType.max,
                )
                allp = const.tile([P, 1], F32)
                nc.gpsimd.partition_all_reduce(
                    allp[:], loc[:], channels=P, reduce_op=bass_isa.ReduceOp.max
                )
                nc.sync.dma_start(out=cc_in[:], in_=allp[0, :])
                nc.gpsimd.collective_compute(
                    "AllReduce",
                    mybir.AluOpType.max,
                    replica_groups=[list(range(N_CORES))],
                    ins=[cc_in[:]],
                    outs=[cc_out[:]],
                )
                g1 = const.tile([1, 1], F32)
                nc.sync.dma_start(out=g1[:], in_=cc_out[:])
                nc.vector.tensor_scalar_max(g1[:], g1[:], 1e-12)
                gb = const.tile([P, 1], F32)
                nc.gpsimd.partition_broadcast(gb[:], g1[:])
                # quant multiplier 224/amax == (448/amax)/2 exactly
                recip = const.tile([P, 1], F32)
                nc.vector.reciprocal(recip[:], gb[:])
                qv = const.tile([P, 1], F32)
                nc.vector.tensor_scalar_mul(qv[:], recip[:], 224.0)
                # dequant multiplier amax*w_scale/112 == 4 * (amax/448) * w_scale
                mf = const.tile([P, 1], F32)
                nc.vector.tensor_mul(out=mf[:], in0=gb[:], in1=wssb[:])
                nc.vector.tensor_scalar_mul(mf[:], mf[:], 1.0 / 112.0)
            else:
                qv = const.tile([P, 1], F32)
                nc.vector.memset(qv[:], 0.125)
                mf = const.tile([P, 1], F32)
                nc.vector.memset(mf[:], 8.0)

            # ---- phase B (reps>1 loops it, for slope benchmarking only) ----
            loop_ctx = tc.For_i(0, reps, 1) if reps > 1 else contextlib.nullcontext()
            with loop_ctx:
                _phase_b(nc, work, xqp, psum, xh, y, wres, qv, mf, dr)

    nc.compile()
    _dedup_ldweights(nc)
    _CACHE[key] = nc
    return nc


def _dedup_ldweights(nc):
    """Drop redundant InstLdweights. tile_legalize splits every matmul
    into LDWEIGHTS+MATMUL with no dedup, so the 4 matmuls that share one
    stationary x_q tile reload it 4 times; the reload (~213 ns, DoubleRow
    loads 256 columns) is on the PE critical path. Deleting a reload is
    safe when it carries no semaphore ops and its weights AP is identical
    to the immediately preceding retained LDWEIGHTS with only matmuls in
    between (walrus pairs each MATMUL with the most recent LDWEIGHTS).
    Verified bitwise-identical outputs; ~7% faster end-to-end.
    """
    removed = 0
    for fn in nc.m.functions:
        for blk in fn.blocks:
            last_sig = None
            keep = []
            for inst in blk.instructions:
                tn = type(inst).__name__
                if tn == "InstLdweights":
                    si = inst.sync_info
                    n_sem = (len(si.on_wait) + len(si.on_update)) if si else 0
                    sig = repr(inst.ins[0])
                    if n_sem == 0 and sig == last_sig:
                        removed += 1
                        continue
                    last_sig = sig
                elif tn != "InstMatmult" and (
                    getattr(inst, "engine", None) == mybir.EngineType.PE
                ):
                    last_sig = None
                keep.append(inst)
            blk.instructions[:] = keep
    return removed


def _pack_inputs(x, weight, w_scale):
    """Host-side shard + layout packing. Returns in_maps for 8 cores."""
    x = np.asarray(x, dtype=np.float32)
    w_fp8fn = np.asarray(weight)
    if w_fp8fn.dtype != E4M3FN:
        w_fp8fn = (
            w_fp8fn.view(E4M3FN) if w_fp8fn.itemsize == 1 else w_fp8fn.astype(E4M3FN)
        )
    ws_val = float(np.asarray(w_scale, dtype=np.float32).reshape(()))

    # x per token-half, packed [TT, P, KO, P]:
    #   [t, p, ko, m] = x[h*T_SH + t*128 + m, ko*128 + p]
    halves = []
    for h in range(TOK_WAYS):
        xhalf = x[h * T_SH : (h + 1) * T_SH]
        packed = np.ascontiguousarray(
            xhalf.reshape(TT, P, KO, P).transpose(0, 3, 2, 1)
        )
        halves.append(packed)

    # W at half scale (exact for fp8 normals), inside TRN fp8's +-240 range
    w_half = (w_fp8fn.astype(np.float32) * 0.5).astype(E4M3FN)
    wqs = []
    for q in range(OF_WAYS):
        wq = w_half[q * OF_SH : (q + 1) * OF_SH]           # [OF_SH, DIN]
        w3 = np.ascontiguousarray(
            wq.T.reshape(KO, P, OF_SH).transpose(1, 0, 2)  # [P, KO, OF_SH]
        ).view(FP8_NP)
        wqs.append(w3)

    ws_arr = np.full((P, 1), ws_val, dtype=np.float32)

    in_maps = []
    for c in range(N_CORES):
        h, q = c // OF_WAYS, c % OF_WAYS
        in_maps.append(
            {
                "xh": halves[h],
                "xa": halves[h][q * AT : (q + 1) * AT],
                "w3": wqs[q],
                "ws": ws_arr,
            }
        )
    return in_maps


def _assemble(results):
    y = np.empty((TOKENS, DOUT), dtype=ml_dtypes.bfloat16)
    for c in range(N_CORES):
        h, q = c // OF_WAYS, c % OF_WAYS
        part = results[c]["y"]
        if part.dtype != ml_dtypes.bfloat16:
            part = part.view(ml_dtypes.bfloat16)
        y[h * T_SH : (h + 1) * T_SH, q * OF_SH : (q + 1) * OF_SH] = part
    return y


def kernel(x, weight, w_scale):
    nc = _build_module()
    in_maps = _pack_inputs(x, weight, w_scale)
    res = run_bass_kernel_spmd(nc, in_maps, list(range(N_CORES)))
    return _assemble(res.results)



# revision 10
# speedup vs baseline: 1.0968x; 1.0968x over previous
"""FP8 linear kernel for Trainium2, 8 NeuronCores.

y = (quant_e4m3fn(x) @ W.T) * (x_inv_scale * w_scale), output bf16.
x [8192, 4096] f32, W [14336, 4096] fp8 e4m3fn, w_scale f32 scalar.

Sharding: 2 token-halves x 4 out_feature-quarters (tensor parallel on
out_features per the hint, plus 2-way data parallel on tokens). Each
core computes y[half, quarter] = [4096, 3584]; the host assembles the
full output.

Exactness strategy: TRN fp8_e4m3 saturates at +-240 (vs OCP e4m3fn's
448), so both operands are staged/quantized at HALF scale (values <=
224), where the two formats agree bit-for-bit, and the dequant factor
carries the compensating 4x. Halving is exact for fp8 normals, so the
kernel reproduces the reference quantization exactly (modulo the fp8
subnormal tail and fp32 summation order).

The global activation amax is computed cooperatively: each core reduces
1/8 of x locally, then an AllReduce(max) collective merges the 8
partial maxima on-device (measured <50us, hidden under the weight-load
DMA).

Matmul: DoubleRow fp8 (2 k-planes per PE cell, K=256 per stationary),
x_qT stationary / W^T moving, PSUM [128 tokens, 4x448] accumulated over
K=4096, double-buffered across the two 1792-wide output halves so
eviction (dequant-scale multiply + bf16 cast on DVE) never stalls the
PE. Host pre-packs x and W^T into partition-major tile layouts so every
DMA is a single fully-contiguous transfer.
"""

import contextlib
import os
import sys

for _p in ("/opt/trn_rl_repo", "/root/.axon_site/_ro/trn_rl_repo"):
    if os.path.isdir(_p) and _p not in sys.path:
        sys.path.insert(0, _p)

import ml_dtypes
import numpy as np

import concourse.bass_isa as bass_isa
import concourse.mybir as mybir
import concourse.tile as tile
from concourse import bacc
from concourse.bass import ds, ts
from concourse.bass_utils import run_bass_kernel_spmd
from concourse.tile_rust import add_dep_helper

F32 = mybir.dt.float32
BF16 = mybir.dt.bfloat16
FP8 = mybir.dt.float8e4
FP8_NP = mybir.dt.np(FP8)
E4M3FN = ml_dtypes.float8_e4m3fn

TOKENS, DIN, DOUT = 8192, 4096, 14336
P = 128
KO = DIN // P               # 32 k-subtiles of 128
N_CORES = 8
TOK_WAYS, OF_WAYS = 2, 4    # sharding grid
T_SH = TOKENS // TOK_WAYS   # 4096 tokens per core
OF_SH = DOUT // OF_WAYS     # 3584 out features per core
TT = T_SH // P              # 32 token tiles per core
AT = TT // OF_WAYS          # 8 amax tiles per core (1/8 of x total)
NB = 512                    # psum bank free dim (full 2KB bank of fp32)
NBANKS = OF_SH // NB        # 7 banks cover all out features in one pass

# 'doublerow' (fast fp8) or 'plain' (exact fp8 at bf16 speed, ~2x slower)
PERF = os.environ.get("FP8LIN_PERF", "doublerow")

_CACHE = {}


def _phase_b(nc, work, xqp, psum, xh, y, wres, qv, mf, dr):
    """Quantize + matmul + scaled eviction over all token tiles.

    Single pass over all OF_SH out features per token tile: 7 PSUM banks
    of 512 (7*512 == 3584), so the x_q stationary is loaded once per
    k2-slab (16 LDWEIGHTS/tile) instead of twice (32 in the two-half
    layout). The psum pool has bufs=8; seven tile() calls per iteration
    rotate through 8 physical banks, so each iteration starts on the one
    bank the previous iteration did NOT use and every bank's eviction
    has ~a full tile of slack before reuse (no WAR stall on the PE).
    """
    for t in range(TT):
        xf = work.tile([P, KO, P], F32, tag="xf")
        nc.sync.dma_start(out=xf[:], in_=xh[t])
        xq = xqp.tile([P, KO, P], FP8, tag="xq")
        nc.scalar.activation(
            xq[:], xf[:], mybir.ActivationFunctionType.Copy, scale=qv[:]
        )
        ps = [
            psum.tile([P, NB], F32, tag="ps", name=f"ps{i}") for i in range(NBANKS)
        ]
        # The explicit edge M(k2,bank0) -> M(k2-1,bank6) pins the greedy
        # Tile scheduler to k2-major order. Without it, the staggered WAR
        # release of psum banks at tile starts (previous tile's evictions
        # retire one by one in the scheduler's pessimistic DVE model) makes
        # it interleave k2 slabs bank-major, which forces a fresh 213ns
        # LDWEIGHTS per matmul instead of one per k2 slab. Same-engine
        # edge = program order on the serial PE, so it costs nothing.
        prev_last = None
        if dr is not None:
            for k2 in range(KO // 2):
                lhsT = xq[:, 2 * k2 : 2 * k2 + 2, :]
                for nb in range(NBANKS):
                    mm = nc.tensor.matmul(
                        ps[nb][:],
                        lhsT,
                        wres[:, 2 * k2 : 2 * k2 + 2, ds(nb * NB, NB)],
                        start=(k2 == 0),
                        stop=(k2 == KO // 2 - 1),
                        perf_mode=dr,
                    )
                    if prev_last is not None:
                        add_dep_helper(mm.ins, prev_last.ins, reason="k2-major PE order")
                prev_last = mm
        else:
            for k in range(KO):
                lhsT = xq[:, k, :]
                for nb in range(NBANKS):
                    mm = nc.tensor.matmul(
                        ps[nb][:],
                        lhsT,
                        wres[:, k, ds(nb * NB, NB)],
                        start=(k == 0),
                        stop=(k == KO - 1),
                    )
                    if prev_last is not None:
                        add_dep_helper(mm.ins, prev_last.ins, reason="k2-major PE order")
                prev_last = mm
        yt = work.tile([P, NBANKS, NB], BF16, tag="yt")
        for nb in range(NBANKS):
            nc.vector.tensor_scalar_mul(yt[:, nb, :], ps[nb][:], mf[:])
        nc.sync.dma_start(out=y[ts(t, P), :], in_=yt[:])


def _build_module(reps=1, phase_a=True):
    """reps>1 loops phase B on-device (slope benchmarking only);
    phase_a=False substitutes constant scales (benchmarking only)."""
    key = ("module", PERF, reps, phase_a)
    if key in _CACHE:
        return _CACHE[key]

    nc = bacc.Bacc(None, target_bir_lowering=False, debug=True, num_devices=N_CORES)
    xh = nc.declare_dram_parameter("xh", [TT, P, KO, P], F32, isOutput=False)
    xa = nc.declare_dram_parameter("xa", [AT, P, KO, P], F32, isOutput=False)
    w3 = nc.declare_dram_parameter("w3", [P, KO, OF_SH], FP8, isOutput=False)
    ws = nc.declare_dram_parameter("ws", [P, 1], F32, isOutput=False)
    y = nc.declare_dram_parameter("y", [T_SH, OF_SH], BF16, isOutput=True)
    cc_in = nc.dram_tensor("cc_in", [1], F32)
    cc_out = nc.dram_tensor("cc_out", [1], F32, addr_space="Shared")

    dr = mybir.MatmulPerfMode.DoubleRow if PERF == "doublerow" else None

    with tile.TileContext(nc) as tc:
        with (
            tc.tile_pool(name="const", bufs=1) as const,
            tc.tile_pool(name="work", bufs=3) as work,
            tc.tile_pool(name="xqp", bufs=2) as xqp,
            tc.tile_pool(name="psum", bufs=8, space="PSUM") as psum,
        ):
            # resident weight [128, KO, OF_SH] fp8 (14.3 MB)
            wres = const.tile([P, KO, OF_SH], FP8)
            for i in range(4):
                nc.sync.dma_start(
                    out=wres[:, ts(i, KO // 4), :], in_=w3[:, ts(i, KO // 4), :]
                )
            wssb = const.tile([P, 1], F32)
            nc.sync.dma_start(out=wssb[:], in_=ws[:])

            if phase_a:
                # ---- phase A: global amax of x ----
                partials = const.tile([P, AT], F32)
                for i in range(AT):
                    xt = work.tile([P, KO, P], F32, tag="xf")
                    nc.sync.dma_start(out=xt[:], in_=xa[i])
                    nc.vector.tensor_reduce(
                        partials[:, i : i + 1],
                        xt[:],
                        axis=mybir.AxisListType.XY,
                        op=mybir.AluOpType.max,
                        apply_absolute_value=True,
                    )
                loc = const.tile([P, 1], F32)
                nc.vector.tensor_reduce(
                    loc[:], partials[:], axis=mybir.AxisListType.X,
                    op=mybir.AluOpType.max,
                )
                allp = const.tile([P, 1], F32)
                nc.gpsimd.partition_all_reduce(
                    allp[:], loc[:], channels=P, reduce_op=bass_isa.ReduceOp.max
                )
                nc.sync.dma_start(out=cc_in[:], in_=allp[0, :])
                nc.gpsimd.collective_compute(
                    "AllReduce",
                    mybir.AluOpType.max,
                    replica_groups=[list(range(N_CORES))],
                    ins=[cc_in[:]],
                    outs=[cc_out[:]],
                )
                g1 = const.tile([1, 1], F32)
                nc.sync.dma_start(out=g1[:], in_=cc_out[:])
                nc.vector.tensor_scalar_max(g1[:], g1[:], 1e-12)
                gb = const.tile([P, 1], F32)
                nc.gpsimd.partition_broadcast(gb[:], g1[:])
                # quant multiplier 224/amax == (448/amax)/2 exactly
                recip = const.tile([P, 1], F32)
                nc.vector.reciprocal(recip[:], gb[:])
                qv = const.tile([P, 1], F32)
                nc.vector.tensor_scalar_mul(qv[:], recip[:], 224.0)
                # dequant multiplier amax*w_scale/112 == 4 * (amax/448) * w_scale
                mf = const.tile([P, 1], F32)
                nc.vector.tensor_mul(out=mf[:], in0=gb[:], in1=wssb[:])
                nc.vector.tensor_scalar_mul(mf[:], mf[:], 1.0 / 112.0)
            else:
                qv = const.tile([P, 1], F32)
                nc.vector.memset(qv[:], 0.125)
                mf = const.tile([P, 1], F32)
                nc.vector.memset(mf[:], 8.0)

            # ---- phase B (reps>1 loops it, for slope benchmarking only) ----
            loop_ctx = tc.For_i(0, reps, 1) if reps > 1 else contextlib.nullcontext()
            with loop_ctx:
                _phase_b(nc, work, xqp, psum, xh, y, wres, qv, mf, dr)

    nc.compile()
    _dedup_ldweights(nc)
    _CACHE[key] = nc
    return nc


def _dedup_ldweights(nc):
    """Drop redundant InstLdweights. tile_legalize splits every matmul
    into LDWEIGHTS+MATMUL with no dedup, so the NBANKS matmuls that share
    one stationary x_q tile reload it each time; the reload (~213 ns,
    DoubleRow loads 256 columns) is on the PE critical path. Deleting a
    reload is safe when its weights AP is identical to the immediately
    preceding retained LDWEIGHTS with only matmuls in between (walrus
    pairs each MATMUL with the most recent LDWEIGHTS). A duplicate that
    carries semaphore WAITS (bacc hoists matmul waits onto the paired
    ldweights) can still be deleted by pushing those waits onto the next
    instruction in program order (the paired matmul) — the wait still
    executes before that matmul issues. Duplicates with on_update are
    kept (their increment is an observable event).
    """
    removed = 0
    for fn in nc.m.functions:
        for blk in fn.blocks:
            last_sig = None
            pending_waits = []
            keep = []
            for inst in blk.instructions:
                tn = type(inst).__name__
                if tn == "InstLdweights":
                    si = inst.sync_info
                    n_upd = len(si.on_update) if si else 0
                    sig = repr(inst.ins[0])
                    if n_upd == 0 and sig == last_sig:
                        if si and si.on_wait:
                            pending_waits.extend(si.on_wait)
                        removed += 1
                        continue
                    last_sig = sig
                elif tn != "InstMatmult" and (
                    getattr(inst, "engine", None) == mybir.EngineType.PE
                ):
                    last_sig = None
                if pending_waits and getattr(inst, "engine", None) == (
                    mybir.EngineType.PE
                ):
                    si = inst.sync_info
                    if si is None:
                        inst.sync_info = mybir.SyncInfo(
                            on_wait=list(pending_waits), on_update=[]
                        )
                    else:
                        si.on_wait = list(si.on_wait) + pending_waits
                    pending_waits = []
                keep.append(inst)
            assert not pending_waits
            blk.instructions[:] = keep
    return removed


def _pack_inputs(x, weight, w_scale):
    """Host-side shard + layout packing. Returns in_maps for 8 cores."""
    x = np.asarray(x, dtype=np.float32)
    w_fp8fn = np.asarray(weight)
    if w_fp8fn.dtype != E4M3FN:
        w_fp8fn = (
            w_fp8fn.view(E4M3FN) if w_fp8fn.itemsize == 1 else w_fp8fn.astype(E4M3FN)
        )
    ws_val = float(np.asarray(w_scale, dtype=np.float32).reshape(()))

    # x per token-half, packed [TT, P, KO, P]:
    #   [t, p, ko, m] = x[h*T_SH + t*128 + m, ko*128 + p]
    halves = []
    for h in range(TOK_WAYS):
        xhalf = x[h * T_SH : (h + 1) * T_SH]
        packed = np.ascontiguousarray(
            xhalf.reshape(TT, P, KO, P).transpose(0, 3, 2, 1)
        )
        halves.append(packed)

    # W at half scale (exact for fp8 normals), inside TRN fp8's +-240 range
    w_half = (w_fp8fn.astype(np.float32) * 0.5).astype(E4M3FN)
    wqs = []
    for q in range(OF_WAYS):
        wq = w_half[q * OF_SH : (q + 1) * OF_SH]           # [OF_SH, DIN]
        w3 = np.ascontiguousarray(
            wq.T.reshape(KO, P, OF_SH).transpose(1, 0, 2)  # [P, KO, OF_SH]
        ).view(FP8_NP)
        wqs.append(w3)

    ws_arr = np.full((P, 1), ws_val, dtype=np.float32)

    in_maps = []
    for c in range(N_CORES):
        h, q = c // OF_WAYS, c % OF_WAYS
        in_maps.append(
            {
                "xh": halves[h],
                "xa": halves[h][q * AT : (q + 1) * AT],
                "w3": wqs[q],
                "ws": ws_arr,
            }
        )
    return in_maps


def _assemble(results):
    y = np.empty((TOKENS, DOUT), dtype=ml_dtypes.bfloat16)
    for c in range(N_CORES):
        h, q = c // OF_WAYS, c % OF_WAYS
        part = results[c]["y"]
        if part.dtype != ml_dtypes.bfloat16:
            part = part.view(ml_dtypes.bfloat16)
        y[h * T_SH : (h + 1) * T_SH, q * OF_SH : (q + 1) * OF_SH] = part
    return y


def kernel(x, weight, w_scale):
    nc = _build_module()
    in_maps = _pack_inputs(x, weight, w_scale)
    res = run_bass_kernel_spmd(nc, in_maps, list(range(N_CORES)))
    return _assemble(res.results)



# revision 13
# speedup vs baseline: 1.2302x; 1.1217x over previous
"""FP8 linear kernel for Trainium2, 8 NeuronCores.

y = (quant_e4m3fn(x) @ W.T) * (x_inv_scale * w_scale), output bf16.
x [8192, 4096] f32, W [14336, 4096] fp8 e4m3fn, w_scale f32 scalar.

Sharding: 2 token-halves x 4 out_feature-quarters (tensor parallel on
out_features per the hint, plus 2-way data parallel on tokens). Each
core computes y[half, quarter] = [4096, 3584]; the host assembles the
full output.

Exactness strategy: TRN fp8_e4m3 saturates at +-240 (vs OCP e4m3fn's
448), so both operands are staged/quantized at HALF scale (values <=
224), where the two formats agree bit-for-bit, and the dequant factor
carries the compensating 4x. Halving is exact for fp8 normals, so the
kernel reproduces the reference quantization exactly (modulo the fp8
subnormal tail and fp32 summation order).

The global activation amax is computed cooperatively: each core reduces
1/8 of x locally, then an AllReduce(max) collective merges the 8
partial maxima on-device (measured <50us, hidden under the weight-load
DMA).

Matmul: DoubleRow fp8 (2 k-planes per PE cell, K=256 per stationary),
x_qT stationary / W^T moving, PSUM [128 tokens, 4x448] accumulated over
K=4096, double-buffered across the two 1792-wide output halves so
eviction (dequant-scale multiply + bf16 cast on DVE) never stalls the
PE. Host pre-packs x and W^T into partition-major tile layouts so every
DMA is a single fully-contiguous transfer.
"""

import contextlib
import os
import sys

for _p in ("/opt/trn_rl_repo", "/root/.axon_site/_ro/trn_rl_repo"):
    if os.path.isdir(_p) and _p not in sys.path:
        sys.path.insert(0, _p)

import ml_dtypes
import numpy as np

import concourse.bass_isa as bass_isa
import concourse.mybir as mybir
import concourse.tile as tile
from concourse import bacc
from concourse.bass import ds, ts
from concourse.bass_utils import run_bass_kernel_spmd
from concourse.tile_rust import add_dep_helper

F32 = mybir.dt.float32
BF16 = mybir.dt.bfloat16
FP8 = mybir.dt.float8e4
FP8_NP = mybir.dt.np(FP8)
E4M3FN = ml_dtypes.float8_e4m3fn

TOKENS, DIN, DOUT = 8192, 4096, 14336
P = 128
KO = DIN // P               # 32 k-subtiles of 128
N_CORES = 8
TOK_WAYS, OF_WAYS = 2, 4    # sharding grid
T_SH = TOKENS // TOK_WAYS   # 4096 tokens per core
OF_SH = DOUT // OF_WAYS     # 3584 out features per core
TT = T_SH // P              # 32 token tiles per core
AT = TT // OF_WAYS          # 8 amax tiles per core (1/8 of x total)
NB = 512                    # psum bank free dim (full 2KB bank of fp32)
NBANKS = OF_SH // NB        # 7 banks cover all out features in one pass

# 'doublerow' (fast fp8), 'swinterleave' (DoubleRowSwInterleave: host
# pre-interleaves the stationary so the weight load reads contiguously),
# or 'plain' (exact fp8 at bf16 speed, ~2x slower)
PERF = os.environ.get("FP8LIN_PERF", "doublerow")
KO2 = KO // 2               # 16 k2 slabs of 256
# per-core packed x tile shape (one 128-token tile): flat 4096 fp8/f32
# per partition either way; swinterleave permutes within each k2 slab
XTILE = [P, KO2, P, 2] if PERF == "swinterleave" else [P, KO, P]

_CACHE = {}


def _phase_b(nc, work, xqp, psum, xh, y, wres, qv, mf, dr):
    """Quantize + matmul + scaled eviction over all token tiles.

    Single pass over all OF_SH out features per token tile: 7 PSUM banks
    of 512 (7*512 == 3584), so the x_q stationary is loaded once per
    k2-slab (16 LDWEIGHTS/tile) instead of twice (32 in the two-half
    layout). The psum pool has bufs=8; seven tile() calls per iteration
    rotate through 8 physical banks, so each iteration starts on the one
    bank the previous iteration did NOT use and every bank's eviction
    has ~a full tile of slack before reuse (no WAR stall on the PE).
    """
    for t in range(TT):
        xf = work.tile(XTILE, F32, tag="xf")
        nc.sync.dma_start(out=xf[:], in_=xh[t])
        xq = xqp.tile(XTILE, FP8, tag="xq")
        nc.scalar.activation(
            xq[:], xf[:], mybir.ActivationFunctionType.Copy, scale=qv[:]
        )
        ps = [
            psum.tile([P, NB], F32, tag="ps", name=f"ps{i}") for i in range(NBANKS)
        ]
        # The explicit edge M(k2,bank0) -> M(k2-1,bank6) pins the greedy
        # Tile scheduler to k2-major order. Without it, the staggered WAR
        # release of psum banks at tile starts (previous tile's evictions
        # retire one by one in the scheduler's pessimistic DVE model) makes
        # it interleave k2 slabs bank-major, which forces a fresh 213ns
        # LDWEIGHTS per matmul instead of one per k2 slab. Same-engine
        # edge = program order on the serial PE, so it costs nothing.
        prev_last = None
        if dr is not None:
            for k2 in range(KO // 2):
                if dr == mybir.MatmulPerfMode.DoubleRowSwInterleave:
                    # host pre-interleaved layout [P, KO2, 128 pairs, 2
                    # planes]; the stationary view is [P, 2, 128] with
                    # steps (1, 2) so the weight read is fully contiguous
                    lhsT = xq[:, k2].rearrange("p q i -> p i q")
                else:
                    lhsT = xq[:, 2 * k2 : 2 * k2 + 2, :]
                for nb in range(NBANKS):
                    mm = nc.tensor.matmul(
                        ps[nb][:],
                        lhsT,
                        wres[:, 2 * k2 : 2 * k2 + 2, ds(nb * NB, NB)],
                        start=(k2 == 0),
                        stop=(k2 == KO // 2 - 1),
                        perf_mode=dr,
                    )
                    if prev_last is not None:
                        add_dep_helper(mm.ins, prev_last.ins, reason="k2-major PE order")
                prev_last = mm
        else:
            for k in range(KO):
                lhsT = xq[:, k, :]
                for nb in range(NBANKS):
                    mm = nc.tensor.matmul(
                        ps[nb][:],
                        lhsT,
                        wres[:, k, ds(nb * NB, NB)],
                        start=(k == 0),
                        stop=(k == KO - 1),
                    )
                    if prev_last is not None:
                        add_dep_helper(mm.ins, prev_last.ins, reason="k2-major PE order")
                prev_last = mm
        yt = work.tile([P, NBANKS, NB], BF16, tag="yt")
        for nb in range(NBANKS):
            nc.vector.tensor_scalar_mul(yt[:, nb, :], ps[nb][:], mf[:])
        nc.sync.dma_start(out=y[ts(t, P), :], in_=yt[:])


def _build_module(reps=1, phase_a=True):
    """reps>1 loops phase B on-device (slope benchmarking only);
    phase_a=False substitutes constant scales (benchmarking only)."""
    key = ("module", PERF, reps, phase_a)
    if key in _CACHE:
        return _CACHE[key]

    nc = bacc.Bacc(None, target_bir_lowering=False, debug=True, num_devices=N_CORES)
    xh = nc.declare_dram_parameter("xh", [TT] + XTILE, F32, isOutput=False)
    xa = nc.declare_dram_parameter("xa", [AT] + XTILE, F32, isOutput=False)
    w3 = nc.declare_dram_parameter("w3", [P, KO, OF_SH], FP8, isOutput=False)
    ws = nc.declare_dram_parameter("ws", [P, 1], F32, isOutput=False)
    y = nc.declare_dram_parameter("y", [T_SH, OF_SH], BF16, isOutput=True)
    cc_in = nc.dram_tensor("cc_in", [1], F32)
    cc_out = nc.dram_tensor("cc_out", [1], F32, addr_space="Shared")

    dr = {
        "doublerow": mybir.MatmulPerfMode.DoubleRow,
        "swinterleave": mybir.MatmulPerfMode.DoubleRowSwInterleave,
    }.get(PERF)

    with tile.TileContext(nc) as tc:
        with (
            tc.tile_pool(name="const", bufs=1) as const,
            tc.tile_pool(name="work", bufs=3) as work,
            tc.tile_pool(name="xqp", bufs=2) as xqp,
            tc.tile_pool(name="psum", bufs=8, space="PSUM") as psum,
        ):
            # resident weight [128, KO, OF_SH] fp8 (14.3 MB)
            wres = const.tile([P, KO, OF_SH], FP8)
            for i in range(4):
                nc.sync.dma_start(
                    out=wres[:, ts(i, KO // 4), :], in_=w3[:, ts(i, KO // 4), :]
                )
            wssb = const.tile([P, 1], F32)
            nc.sync.dma_start(out=wssb[:], in_=ws[:])

            if phase_a:
                # ---- phase A: global amax of x ----
                partials = const.tile([P, AT], F32)
                for i in range(AT):
                    xt = work.tile(XTILE, F32, tag="xf")
                    nc.sync.dma_start(out=xt[:], in_=xa[i])
                    xt_v = (
                        xt.rearrange("p a b c -> p a (b c)")
                        if len(XTILE) == 4
                        else xt[:]
                    )
                    nc.vector.tensor_reduce(
                        partials[:, i : i + 1],
                        xt_v,
                        axis=mybir.AxisListType.XY,
                        op=mybir.AluOpType.max,
                        apply_absolute_value=True,
                    )
                loc = const.tile([P, 1], F32)
                nc.vector.tensor_reduce(
                    loc[:], partials[:], axis=mybir.AxisListType.X,
                    op=mybir.AluOpType.max,
                )
                allp = const.tile([P, 1], F32)
                nc.gpsimd.partition_all_reduce(
                    allp[:], loc[:], channels=P, reduce_op=bass_isa.ReduceOp.max
                )
                nc.sync.dma_start(out=cc_in[:], in_=allp[0, :])
                nc.gpsimd.collective_compute(
                    "AllReduce",
                    mybir.AluOpType.max,
                    replica_groups=[list(range(N_CORES))],
                    ins=[cc_in[:]],
                    outs=[cc_out[:]],
                )
                g1 = const.tile([1, 1], F32)
                nc.sync.dma_start(out=g1[:], in_=cc_out[:])
                nc.vector.tensor_scalar_max(g1[:], g1[:], 1e-12)
                gb = const.tile([P, 1], F32)
                nc.gpsimd.partition_broadcast(gb[:], g1[:])
                # quant multiplier 224/amax == (448/amax)/2 exactly
                recip = const.tile([P, 1], F32)
                nc.vector.reciprocal(recip[:], gb[:])
                qv = const.tile([P, 1], F32)
                nc.vector.tensor_scalar_mul(qv[:], recip[:], 224.0)
                # dequant multiplier amax*w_scale/112 == 4 * (amax/448) * w_scale
                mf = const.tile([P, 1], F32)
                nc.vector.tensor_mul(out=mf[:], in0=gb[:], in1=wssb[:])
                nc.vector.tensor_scalar_mul(mf[:], mf[:], 1.0 / 112.0)
            else:
                qv = const.tile([P, 1], F32)
                nc.vector.memset(qv[:], 0.125)
                mf = const.tile([P, 1], F32)
                nc.vector.memset(mf[:], 8.0)

            # ---- phase B (reps>1 loops it, for slope benchmarking only) ----
            loop_ctx = tc.For_i(0, reps, 1) if reps > 1 else contextlib.nullcontext()
            with loop_ctx:
                _phase_b(nc, work, xqp, psum, xh, y, wres, qv, mf, dr)

    nc.compile()
    _dedup_ldweights(nc)
    _CACHE[key] = nc
    return nc


def _dedup_ldweights(nc):
    """Drop redundant InstLdweights. tile_legalize splits every matmul
    into LDWEIGHTS+MATMUL with no dedup, so the NBANKS matmuls that share
    one stationary x_q tile reload it each time; the reload (~213 ns,
    DoubleRow loads 256 columns) is on the PE critical path. Deleting a
    reload is safe when its weights AP is identical to the immediately
    preceding retained LDWEIGHTS with only matmuls in between (walrus
    pairs each MATMUL with the most recent LDWEIGHTS). A duplicate that
    carries semaphore WAITS (bacc hoists matmul waits onto the paired
    ldweights) can still be deleted by pushing those waits onto the next
    instruction in program order (the paired matmul) — the wait still
    executes before that matmul issues. Duplicates with on_update are
    kept (their increment is an observable event).
    """
    removed = 0
    for fn in nc.m.functions:
        for blk in fn.blocks:
            last_sig = None
            pending_waits = []
            keep = []
            for inst in blk.instructions:
                tn = type(inst).__name__
                if tn == "InstLdweights":
                    si = inst.sync_info
                    n_upd = len(si.on_update) if si else 0
                    sig = repr(inst.ins[0])
                    if n_upd == 0 and sig == last_sig:
                        if si and si.on_wait:
                            pending_waits.extend(si.on_wait)
                        removed += 1
                        continue
                    last_sig = sig
                elif tn != "InstMatmult" and (
                    getattr(inst, "engine", None) == mybir.EngineType.PE
                ):
                    last_sig = None
                if pending_waits and getattr(inst, "engine", None) == (
                    mybir.EngineType.PE
                ):
                    si = inst.sync_info
                    if si is None:
                        inst.sync_info = mybir.SyncInfo(
                            on_wait=list(pending_waits), on_update=[]
                        )
                    else:
                        si.on_wait = list(si.on_wait) + pending_waits
                    pending_waits = []
                keep.append(inst)
            assert not pending_waits
            blk.instructions[:] = keep
    return removed


def _pack_inputs(x, weight, w_scale):
    """Host-side shard + layout packing. Returns in_maps for 8 cores."""
    x = np.asarray(x, dtype=np.float32)
    w_fp8fn = np.asarray(weight)
    if w_fp8fn.dtype != E4M3FN:
        w_fp8fn = (
            w_fp8fn.view(E4M3FN) if w_fp8fn.itemsize == 1 else w_fp8fn.astype(E4M3FN)
        )
    ws_val = float(np.asarray(w_scale, dtype=np.float32).reshape(()))

    # x per token-half. doublerow/plain pack [TT, P, KO, P]:
    #   [t, p, ko, m] = x[h*T_SH + t*128 + m, ko*128 + p]
    # swinterleave packs [TT, P, KO2, 128, 2] with the token axis reversed
    # and the two k-planes of each k2 slab interleaved per pair -- the
    # physical order DoubleRow's hardware weight load would otherwise
    # gather, so the LDWEIGHTS read is one contiguous 256B/partition run:
    #   [t, p, k2, q, i] = x[h*T_SH + t*128 + (127-q), (2*k2+i)*128 + p]
    halves = []
    for h in range(TOK_WAYS):
        xhalf = x[h * T_SH : (h + 1) * T_SH]
        if PERF == "swinterleave":
            arr = xhalf.reshape(TT, P, KO2, 2, P)   # [t, m, k2, i, p]
            packed = np.ascontiguousarray(
                arr[:, ::-1].transpose(0, 4, 2, 1, 3)
            )
        else:
            packed = np.ascontiguousarray(
                xhalf.reshape(TT, P, KO, P).transpose(0, 3, 2, 1)
            )
        halves.append(packed)

    # W at half scale (exact for fp8 normals), inside TRN fp8's +-240 range
    w_half = (w_fp8fn.astype(np.float32) * 0.5).astype(E4M3FN)
    wqs = []
    for q in range(OF_WAYS):
        wq = w_half[q * OF_SH : (q + 1) * OF_SH]           # [OF_SH, DIN]
        w3 = np.ascontiguousarray(
            wq.T.reshape(KO, P, OF_SH).transpose(1, 0, 2)  # [P, KO, OF_SH]
        ).view(FP8_NP)
        wqs.append(w3)

    ws_arr = np.full((P, 1), ws_val, dtype=np.float32)

    in_maps = []
    for c in range(N_CORES):
        h, q = c // OF_WAYS, c % OF_WAYS
        in_maps.append(
            {
                "xh": halves[h],
                "xa": halves[h][q * AT : (q + 1) * AT],
                "w3": wqs[q],
                "ws": ws_arr,
            }
        )
    return in_maps


def _assemble(results):
    y = np.empty((TOKENS, DOUT), dtype=ml_dtypes.bfloat16)
    for c in range(N_CORES):
        h, q = c // OF_WAYS, c % OF_WAYS
        part = results[c]["y"]
        if part.dtype != ml_dtypes.bfloat16:
            part = part.view(ml_dtypes.bfloat16)
        y[h * T_SH : (h + 1) * T_SH, q * OF_SH : (q + 1) * OF_SH] = part
    return y


def kernel(x, weight, w_scale):
    nc = _build_module()
    in_maps = _pack_inputs(x, weight, w_scale)
    res = run_bass_kernel_spmd(nc, in_maps, list(range(N_CORES)))
    return _assemble(res.results)

